# revision 58
# baseline (speedup 1.0000x reference)
"""CrossFeatureAttention TRN2 kernel (fp8 DoubleRow, folded projections).

Full inputs -> full output. Sharding: data-parallel over (batch b, half of N1)
across 8 cores; each core computes out[b, h*2048:(h+1)*2048, :].

Math per core (q=2048 rows of x1, x2[b] 4096 rows, C=512), using
associativity to avoid projecting K and V over the long N2 axis:

    Q   = x1 @ Wq^T + (bq + bv)          (bf16 matmul; fp32 kept for residual)
    Q'  = Q @ Wk                         (fp8 DR; Q.bk is constant per row and
                                          cancels in softmax, so bk is dropped)
    S^T = x2t^T-pairs . Q'               (fp8 DR)  == scores^T
    P   = exp(S / sqrt(C))               (ACT -> fp8)
    rs  = 16 * colsum(P^T)               (DR matmul with 16.0-constant lhsT)
    A'  = P @ x2                         (fp8 DR, scaled x16 into fp8)
    att = A' @ Wv^T                      (fp8 DR over the short C axis)
    U^T = Q^T + att^T * recip(rs)        (x16 and /16 cancel; rs*recip==1
                                          also makes the +bv fold exact)
    out = U @ Wo^T + bo                  (f32r matmul, full rate, high prec)

All fp8 matmuls use MatmulPerfMode.DoubleRow with operands holding
contraction k-tile pairs in [128, 2, F] layout (2 cols/cycle).  Per q-chunk
of 512 rows, rowsum/A' groups are interleaved into the S loop with a lag so
the ACT exp stream is never on the PE critical path.
"""

import os
import sys

import numpy as np

for _p in ("/root/.axon_site", "/root/.axon_site/_ro/trn_rl_repo",
           "/root/.axon_site/_ro/pypackages"):
    if _p not in sys.path and os.path.isdir(_p):
        sys.path.append(_p)

import ml_dtypes

import concourse.bacc as bacc
import concourse.mybir as mybir
import concourse.tile as tile
from concourse.bass_utils import run_bass_kernel_spmd

F32 = mybir.dt.float32
F32R = mybir.dt.float32r
BF16 = mybir.dt.bfloat16
F8 = mybir.dt.float8e4
AF = mybir.ActivationFunctionType
DR = mybir.MatmulPerfMode.DoubleRow

B, N1, N2, C = 4, 4096, 4096, 512
NCORES = 8
QROWS = N1 * B // NCORES          # 2048 q rows per core
QC = 512                          # q-chunk
NQC = QROWS // QC                 # 4 chunks
KT = N2 // 128                    # 32 k-tiles
CCH = C // 128                    # 4 contraction chunks
SCALE = 1.0 / float(np.sqrt(C))
A_SCALE = 0.25                    # keeps unnormalized A' inside fp8 range

_BUILT = None


def build():
    nc = bacc.Bacc(None, target_bir_lowering=False, debug=False)

    x1t_d = nc.dram_tensor("x1t", [128, CCH, QROWS], BF16, kind="ExternalInput")
    x2t_d = nc.dram_tensor("x2t", [128, CCH, N2], F8, kind="ExternalInput")
    x2p_d = nc.dram_tensor("x2p", [128, KT, C], F8, kind="ExternalInput")
    wq_d = nc.dram_tensor("wq8", [128, CCH, C], BF16, kind="ExternalInput")
    wkp_d = nc.dram_tensor("wkp", [128, CCH, C], F8, kind="ExternalInput")
    wv_d = nc.dram_tensor("wv8", [128, CCH, C], F8, kind="ExternalInput")
    wo_d = nc.dram_tensor("wo8", [128, CCH, C], F32, kind="ExternalInput")
    bq_d = nc.dram_tensor("bq", [128, CCH], F32, kind="ExternalInput")
    bo_d = nc.dram_tensor("bo", [C], F32, kind="ExternalInput")
    out_d = nc.dram_tensor("out", [NQC * 4, 128, C], F32, kind="ExternalOutput")

    with tile.TileContext(nc) as tc:
        with tc.tile_pool(name="cst", bufs=1) as cst, \
             tc.tile_pool(name="per", bufs=1) as per, \
             tc.tile_pool(name="sb", bufs=1) as sb, \
             tc.tile_pool(name="ps", bufs=1, space="PSUM") as ps:

            # ---- constants / weights (batched DMAs, needed-first order) ----
            s16 = cst.tile([128, 2, 128], F8)
            nc.gpsimd.memset(s16[:], A_SCALE)
            warm = cst.tile([128, 2, 512], F8, name="warm", tag="warm")
            nc.gpsimd.memset(warm[:], 0.0)

            wq_b = cst.tile([128, CCH, C], BF16, name="wqb", tag="wqb")
            nc.sync.dma_start(out=wq_b[:], in_=wq_d[:])
            x1t = cst.tile([128, CCH, QROWS], BF16, name="x1tb", tag="x1tb")
            nc.sync.dma_start(out=x1t[:, :, 0:QC], in_=x1t_d[:, :, 0:QC])
            bq_b = cst.tile([128, CCH], F32, name="bqb", tag="bqb")
            nc.sync.dma_start(out=bq_b[:], in_=bq_d[:])
            bq_t = [bq_b[:, d:d + 1] for d in range(CCH)]
            wkp = cst.tile([128, CCH, C], F8, name="wkp", tag="wkp")
            nc.sync.dma_start(out=wkp[:], in_=wkp_d[:])

            x2t = per.tile([128, CCH, N2], F8, name="x2t", tag="x2t")
            nc.sync.dma_start(out=x2t[:, :, 0:1024], in_=x2t_d[:, :, 0:1024])
            x2p = per.tile([128, KT, C], F8, name="x2p", tag="x2p")
            nc.sync.dma_start(out=x2p[:, 0:KT // 2, :], in_=x2p_d[:, 0:KT // 2, :])
            nc.sync.dma_start(out=x2t[:, :, 1024:2048], in_=x2t_d[:, :, 1024:2048])
            nc.sync.dma_start(out=x2p[:, KT // 2:KT, :], in_=x2p_d[:, KT // 2:KT, :])
            nc.sync.dma_start(out=x2t[:, :, 2048:4096], in_=x2t_d[:, :, 2048:4096])
            nc.sync.dma_start(out=x1t[:, :, QC:QROWS], in_=x1t_d[:, :, QC:QROWS])
            wv8 = cst.tile([128, CCH, C], F8, name="wv8", tag="wv8")
            nc.sync.dma_start(out=wv8[:], in_=wv_d[:])

            wo_stage = sb.tile([128, CCH, C], F32, name="wos", tag="wos", bufs=1)
            nc.sync.dma_start(out=wo_stage[:], in_=wo_d[:])
            wo_t = [cst.tile([128, C], F32R, name=f"wo{d}", tag=f"wo{d}")
                    for d in range(CCH)]
            bo_bc = cst.tile([128, C], F32)
            nc.sync.dma_start(out=bo_bc[:], in_=bo_d[:].unsqueeze(0).broadcast_to([128, C]))

            # ---- per-chunk state (double buffered across chunks) ----
            def chunk_tiles():
                return {
                    "qtf": [sb.tile([128, QC], F32, name=f"qtf{d}", tag=f"qtf{d}",
                                    bufs=2) for d in range(CCH)],
                    "qt8": sb.tile([128, CCH, QC], F8, name="qt8", tag="qt8", bufs=2),
                    "qp8": sb.tile([128, CCH, QC], F8, name="qp8", tag="qp8", bufs=2),
                    "a8": sb.tile([128, CCH, QC], F8, name="a8", tag="a8", bufs=2),
                    "pt8": [sb.tile([128, 2, QC], F8, name=f"pt{j}", tag=f"pt{j}",
                                    bufs=2) for j in range(KT // 2)],
                    "recip": sb.tile([128, QC], F32, name="recip", tag="recip", bufs=2),
                    "u": [sb.tile([128, QC], F32R, name=f"u{d}", tag=f"u{d}", bufs=2)
                          for d in range(CCH)],
                }

            st = [None] * NQC

            def emit_q(i):
                # Q^T (bf16, fp32 out) with bias bq+bv; fp8 copy for Q'
                st[i] = chunk_tiles()
                q0 = i * QC
                for d in range(CCH):
                    qp = ps.tile([128, QC], F32, name="qp", tag="pR", bufs=2)
                    for cc in range(CCH):
                        nc.tensor.matmul(qp[:],
                                         lhsT=wq_b[:, cc, d * 128:(d + 1) * 128],
                                         rhs=x1t[:, cc, q0:q0 + QC],
                                         start=(cc == 0), stop=(cc == CCH - 1))
                    nc.vector.tensor_add(out=st[i]["qtf"][d][:], in0=qp[:],
                                         in1=bq_t[d][:].broadcast_to([128, QC]))
                    nc.scalar.activation(st[i]["qt8"][:, d, :], qp[:], AF.Identity,
                                         bias=bq_t[d][:])

            def emit_qprime(i):
                # Q'^T[c,q] = sum_d Wk[d,c] Q^T[d,q]  (fp8 DR over d-pairs)
                for cch in range(CCH):
                    pp = ps.tile([128, QC], F32, name="qpp", tag="pR", bufs=2)
                    for j2 in range(2):
                        nc.tensor.matmul(
                            pp[:],
                            lhsT=wkp[:, 2 * j2:2 * j2 + 2, cch * 128:(cch + 1) * 128],
                            rhs=st[i]["qt8"][:, 2 * j2:2 * j2 + 2, :],
                            start=(j2 == 0), stop=(j2 == 1), perf_mode=DR)
                    nc.scalar.activation(st[i]["qp8"][:, cch, :], pp[:], AF.Copy)

            def emit_s_kt(i, kt):
                # S^T[k-tile, q] = sum_c x2^T[c,k] Q'^T[c,q]; exp -> fp8
                sp = ps.tile([128, QC], F32, name="sps", tag="pA", bufs=2)
                for j2 in range(2):
                    nc.tensor.matmul(
                        sp[:],
                        lhsT=x2t[:, 2 * j2:2 * j2 + 2, kt * 128:(kt + 1) * 128],
                        rhs=st[i]["qp8"][:, 2 * j2:2 * j2 + 2, :],
                        start=(j2 == 0), stop=(j2 == 1), perf_mode=DR)
                nc.scalar.activation(st[i]["pt8"][kt // 2][:, kt % 2, :], sp[:],
                                     AF.Exp, scale=float(SCALE))

            def emit_r_j(i, j, rs):
                nc.tensor.matmul(rs[:], lhsT=s16[:], rhs=st[i]["pt8"][j][:],
                                 start=(j == 0), stop=(j == KT // 2 - 1),
                                 perf_mode=DR)

            def emit_ap_j(i, j, ap):
                # A'^T[c,q] += x2[k,c-block]^T-pairs . P^T[k,q]
                for cch in range(CCH):
                    nc.tensor.matmul(ap[cch // 2][:, cch % 2, :],
                                     lhsT=x2p[:, 2 * j:2 * j + 2,
                                              cch * 128:(cch + 1) * 128],
                                     rhs=st[i]["pt8"][j][:],
                                     start=(j == 0), stop=(j == KT // 2 - 1),
                                     perf_mode=DR)

            LAG = 3

            def emit_o_rb(i, rb):
                op = ps.tile([128, C], F32, name="ops", tag="pR", bufs=2)
                for d in range(CCH):
                    nc.tensor.matmul(
                        op[:],
                        lhsT=st[i]["u"][d][:, rb * 128:(rb + 1) * 128],
                        rhs=wo_t[d][:],
                        start=(d == 0), stop=(d == CCH - 1))
                ot = sb.tile([128, C], F32, name="ot", tag="ot", bufs=3)
                nc.vector.tensor_add(out=ot[:], in0=op[:], in1=bo_bc[:])
                nc.sync.dma_start(out=out_d[i * 4 + rb, :, :], in_=ot[:])

            def emit_chunk_core(i, o_prev=None):
                # S loop with rowsum/A' groups lagged in behind the exps;
                # the previous chunk's output projection fills the lag prefix
                rs = ps.tile([128, QC], F32, name="rs", tag="pR", bufs=2)
                ap = [ps.tile([128, 2, QC], F32, name="aps", tag="pB", bufs=2)
                      for _ in range(CCH)]
                for j in range(KT // 2):
                    emit_s_kt(i, 2 * j)
                    emit_s_kt(i, 2 * j + 1)
                    if o_prev is not None and 1 <= j <= 4:
                        emit_o_rb(o_prev, j - 1)
                    elif o_prev is None and 1 <= j <= 4:
                        # chunk 0 has no prior output projection to fill the
                        # lag prefix; warm matmuls keep PE busy while the ACT
                        # exp stream ramps
                        for w in range(2):
                            wp = ps.tile([128, QC], F32, name="warmp",
                                         tag="pR", bufs=2)
                            nc.tensor.matmul(wp[:], lhsT=s16[:], rhs=warm[:],
                                             start=True, stop=True,
                                             perf_mode=DR)
                    if j >= LAG:
                        emit_r_j(i, j - LAG, rs)
                        emit_ap_j(i, j - LAG, ap)
                for j in range(KT // 2 - LAG, KT // 2):
                    emit_r_j(i, j, rs)
                    emit_ap_j(i, j, ap)
                # scaled fp8 copies of A' (x0.25 keeps it inside fp8 range;
                # the scales in rs and a8 cancel through recip) — one wide
                # ACT op per 2-bank accumulator halves the cast overhead
                for h in range(2):
                    nc.scalar.activation(st[i]["a8"][:, 2 * h:2 * h + 2, :],
                                         ap[h][:], AF.Copy,
                                         scale=float(A_SCALE))
                nc.vector.reciprocal_approx_fast(out=st[i]["recip"][:], in_=rs[:])

            def emit_att_u(i):
                # att^T[d,q] = sum_c Wv^T[c,d] A'^T[c,q]; U = Q + att*recip
                for h in range(2):
                    pp = ps.tile([128, 2, QC], F32, name="attp", tag="pB",
                                 bufs=2)
                    for i2 in range(2):
                        d = 2 * h + i2
                        for j2 in range(2):
                            nc.tensor.matmul(
                                pp[:, i2, :],
                                lhsT=wv8[:, 2 * j2:2 * j2 + 2,
                                         d * 128:(d + 1) * 128],
                                rhs=st[i]["a8"][:, 2 * j2:2 * j2 + 2, :],
                                start=(j2 == 0), stop=(j2 == 1), perf_mode=DR)
                    for i2 in range(2):
                        d = 2 * h + i2
                        at = sb.tile([128, QC], F32, name="at", tag="at",
                                     bufs=2)
                        nc.vector.tensor_mul(out=at[:], in0=pp[:, i2, :],
                                             in1=st[i]["recip"][:])
                        nc.vector.tensor_add(out=st[i]["u"][d][:], in0=at[:],
                                             in1=st[i]["qtf"][d][:])

            # ---- schedule ----
            # PE warmup during the DMA lead-in (clock ramp + covers the
            # x1/wq transfer before Q0 can start)
            def emit_warm(n):
                for w in range(n):
                    wp = ps.tile([128, QC], F32, name="warmp", tag="pA", bufs=2)
                    nc.tensor.matmul(wp[:], lhsT=s16[:], rhs=warm[:],
                                     start=True, stop=True, perf_mode=DR)

            emit_warm(11)
            emit_q(0)
            emit_qprime(0)
            emit_warm(3)
            for i in range(NQC):
                emit_chunk_core(i, o_prev=(i - 1 if i > 0 else None))
                if i == 0:
                    for d in range(CCH):
                        nc.gpsimd.tensor_copy(out=wo_t[d][:],
                                              in_=wo_stage[:, d, :])
                if i + 1 < NQC:
                    emit_q(i + 1)
                emit_att_u(i)
                if i + 1 < NQC:
                    emit_qprime(i + 1)
            for rb in range(QC // 128):
                emit_o_rb(NQC - 1, rb)

    nc.compile()
    return nc


def get_built():
    global _BUILT
    if _BUILT is None:
        _BUILT = build()
    return _BUILT


def _pair_layout(a, dt):
    # [512 (contract), X] -> [128, 4, X]: [p, j, x] = a[j*128+p, x]
    return np.ascontiguousarray(
        a.reshape(CCH, 128, -1).transpose(1, 0, 2)).astype(dt)


def make_in_maps(x1, x2, Wq, bq, Wk, bk, Wv, bv, Wo, bo):
    bf = ml_dtypes.bfloat16
    f8 = ml_dtypes.float8_e4m3
    wq8 = _pair_layout(np.ascontiguousarray(Wq.T), bf)
    wkp = _pair_layout(np.ascontiguousarray(Wk), f8)
    wv8 = _pair_layout(np.ascontiguousarray(Wv.T), f8)
    wo8 = _pair_layout(np.ascontiguousarray(Wo.T), np.float32)
    # bv folds into the Q bias (rs*recip == 1); bk cancels in softmax
    bqv = (bq + bv).astype(np.float32)
    bq32 = np.ascontiguousarray(bqv.reshape(CCH, 128).T).astype(np.float32)
    bo32 = bo.astype(np.float32)
    x2t8 = [_pair_layout(np.ascontiguousarray(x2[b].T), f8) for b in range(B)]
    x2p8 = [np.ascontiguousarray(
        x2[b].reshape(KT, 128, C).transpose(1, 0, 2)).astype(f8)
        for b in range(B)]
    in_maps = []
    for cid in range(NCORES):
        b, h = cid // 2, cid % 2
        x1s = x1[b, h * QROWS:(h + 1) * QROWS, :]
        in_maps.append({
            "x1t": _pair_layout(np.ascontiguousarray(x1s.T), bf),
            "x2t": x2t8[b], "x2p": x2p8[b],
            "wq8": wq8, "wkp": wkp, "wv8": wv8, "wo8": wo8,
            "bq": bq32, "bo": bo32,
        })
    return in_maps


LAST_RESULT = None


def kernel(x1, x2, Wq, bq, Wk, bk, Wv, bv, Wo, bo):
    global LAST_RESULT
    nc = get_built()
    in_maps = make_in_maps(x1, x2, Wq, bq, Wk, bk, Wv, bv, Wo, bo)
    trace = bool(os.environ.get("KERNEL_TRACE"))
    res = run_bass_kernel_spmd(nc, in_maps, core_ids=list(range(NCORES)), trace=trace)
    LAST_RESULT = res
    out = np.empty((B, N1, C), dtype=np.float32)
    for cid in range(NCORES):
        b, h = cid // 2, cid % 2
        out[b, h * QROWS:(h + 1) * QROWS, :] = \
            res.results[cid]["out"].reshape(QROWS, C)
    return out


# revision 59
# speedup vs baseline: 1.0147x; 1.0147x over previous
"""CrossFeatureAttention TRN2 kernel (fp8 DoubleRow, folded projections).

Full inputs -> full output. Sharding: data-parallel over (batch b, half of N1)
across 8 cores; each core computes out[b, h*2048:(h+1)*2048, :].

Math per core (q=2048 rows of x1, x2[b] 4096 rows, C=512), using
associativity to avoid projecting K and V over the long N2 axis:

    Q   = x1 @ Wq^T + (bq + bv)          (bf16 matmul; fp32 kept for residual)
    Q'  = Q @ Wk                         (fp8 DR; Q.bk is constant per row and
                                          cancels in softmax, so bk is dropped)
    S^T = x2t^T-pairs . Q'               (fp8 DR)  == scores^T
    P   = exp(S / sqrt(C))               (ACT -> fp8)
    rs  = 16 * colsum(P^T)               (DR matmul with 16.0-constant lhsT)
    A'  = P @ x2                         (fp8 DR, scaled x16 into fp8)
    att = A' @ Wv^T                      (fp8 DR over the short C axis)
    U^T = Q^T + att^T * recip(rs)        (x16 and /16 cancel; rs*recip==1
                                          also makes the +bv fold exact)
    out = U @ Wo^T + bo                  (f32r matmul, full rate, high prec)

All fp8 matmuls use MatmulPerfMode.DoubleRow with operands holding
contraction k-tile pairs in [128, 2, F] layout (2 cols/cycle).  Per q-chunk
of 512 rows, rowsum/A' groups are interleaved into the S loop with a lag so
the ACT exp stream is never on the PE critical path.
"""

import os
import sys

import numpy as np

for _p in ("/root/.axon_site", "/root/.axon_site/_ro/trn_rl_repo",
           "/root/.axon_site/_ro/pypackages"):
    if _p not in sys.path and os.path.isdir(_p):
        sys.path.append(_p)

import ml_dtypes

import concourse.bacc as bacc
import concourse.mybir as mybir
import concourse.tile as tile
from concourse.bass_utils import run_bass_kernel_spmd

F32 = mybir.dt.float32
F32R = mybir.dt.float32r
BF16 = mybir.dt.bfloat16
F8 = mybir.dt.float8e4
AF = mybir.ActivationFunctionType
DR = mybir.MatmulPerfMode.DoubleRow

B, N1, N2, C = 4, 4096, 4096, 512
NCORES = 8
QROWS = N1 * B // NCORES          # 2048 q rows per core
QC = 512                          # q-chunk
NQC = QROWS // QC                 # 4 chunks
KT = N2 // 128                    # 32 k-tiles
CCH = C // 128                    # 4 contraction chunks
SCALE = 1.0 / float(np.sqrt(C))
A_SCALE = 0.25                    # keeps unnormalized A' inside fp8 range

_BUILT = None


def build():
    nc = bacc.Bacc(None, target_bir_lowering=False, debug=False)

    x1t_d = nc.dram_tensor("x1t", [128, CCH, QROWS], BF16, kind="ExternalInput")
    x2t_d = nc.dram_tensor("x2t", [128, CCH, N2], F8, kind="ExternalInput")
    x2p_d = nc.dram_tensor("x2p", [128, KT, C], F8, kind="ExternalInput")
    wq_d = nc.dram_tensor("wq8", [128, CCH, C], BF16, kind="ExternalInput")
    wkp_d = nc.dram_tensor("wkp", [128, CCH, C], F8, kind="ExternalInput")
    wv_d = nc.dram_tensor("wv8", [128, CCH, C], F8, kind="ExternalInput")
    wo_d = nc.dram_tensor("wo8", [128, CCH, C], F32, kind="ExternalInput")
    bq_d = nc.dram_tensor("bq", [128, CCH], F32, kind="ExternalInput")
    bo_d = nc.dram_tensor("bo", [C], F32, kind="ExternalInput")
    out_d = nc.dram_tensor("out", [NQC * 4, 128, C], F32, kind="ExternalOutput")

    with tile.TileContext(nc) as tc:
        with tc.tile_pool(name="cst", bufs=1) as cst, \
             tc.tile_pool(name="per", bufs=1) as per, \
             tc.tile_pool(name="sb", bufs=1) as sb, \
             tc.tile_pool(name="ps", bufs=1, space="PSUM") as ps:

            # ---- constants / weights (batched DMAs, needed-first order) ----
            s16 = cst.tile([128, 2, 128], F8)
            nc.gpsimd.memset(s16[:], A_SCALE)
            warm = cst.tile([128, 2, 512], F8, name="warm", tag="warm")
            nc.gpsimd.memset(warm[:], 0.0)

            wq_b = cst.tile([128, CCH, C], BF16, name="wqb", tag="wqb")
            nc.sync.dma_start(out=wq_b[:], in_=wq_d[:])
            x1t = cst.tile([128, CCH, QROWS], BF16, name="x1tb", tag="x1tb")
            nc.sync.dma_start(out=x1t[:, :, 0:QC], in_=x1t_d[:, :, 0:QC])
            bq_b = cst.tile([128, CCH], F32, name="bqb", tag="bqb")
            nc.sync.dma_start(out=bq_b[:], in_=bq_d[:])
            bq_t = [bq_b[:, d:d + 1] for d in range(CCH)]
            wkp = cst.tile([128, CCH, C], F8, name="wkp", tag="wkp")
            nc.sync.dma_start(out=wkp[:], in_=wkp_d[:])

            x2t = per.tile([128, CCH, N2], F8, name="x2t", tag="x2t")
            nc.sync.dma_start(out=x2t[:, :, 0:1024], in_=x2t_d[:, :, 0:1024])
            x2p = per.tile([128, KT, C], F8, name="x2p", tag="x2p")
            nc.sync.dma_start(out=x2p[:, 0:KT // 2, :], in_=x2p_d[:, 0:KT // 2, :])
            nc.sync.dma_start(out=x2t[:, :, 1024:2048], in_=x2t_d[:, :, 1024:2048])
            nc.sync.dma_start(out=x2p[:, KT // 2:KT, :], in_=x2p_d[:, KT // 2:KT, :])
            nc.sync.dma_start(out=x2t[:, :, 2048:4096], in_=x2t_d[:, :, 2048:4096])
            nc.sync.dma_start(out=x1t[:, :, QC:QROWS], in_=x1t_d[:, :, QC:QROWS])
            wv8 = cst.tile([128, CCH, C], F8, name="wv8", tag="wv8")
            nc.sync.dma_start(out=wv8[:], in_=wv_d[:])

            wo_stage = sb.tile([128, CCH, C], F32, name="wos", tag="wos", bufs=1)
            nc.sync.dma_start(out=wo_stage[:], in_=wo_d[:])
            wo_t = [cst.tile([128, C], F32R, name=f"wo{d}", tag=f"wo{d}")
                    for d in range(CCH)]
            bo_bc = cst.tile([128, C], F32)
            nc.sync.dma_start(out=bo_bc[:], in_=bo_d[:].unsqueeze(0).broadcast_to([128, C]))

            # ---- per-chunk state (double buffered across chunks) ----
            def chunk_tiles():
                return {
                    "qtf": [sb.tile([128, QC], F32, name=f"qtf{d}", tag=f"qtf{d}",
                                    bufs=2) for d in range(CCH)],
                    "qt8": sb.tile([128, CCH, QC], F8, name="qt8", tag="qt8", bufs=2),
                    "qp8": sb.tile([128, CCH, QC], F8, name="qp8", tag="qp8", bufs=2),
                    "a8": sb.tile([128, CCH, QC], F8, name="a8", tag="a8", bufs=2),
                    "pt8": [sb.tile([128, 2, QC], F8, name=f"pt{j}", tag=f"pt{j}",
                                    bufs=2) for j in range(KT // 2)],
                    "recip": sb.tile([128, QC], F32, name="recip", tag="recip", bufs=2),
                    "u": [sb.tile([128, QC], F32R, name=f"u{d}", tag=f"u{d}", bufs=2)
                          for d in range(CCH)],
                }

            st = [None] * NQC

            def emit_q(i):
                # Q^T (bf16, fp32 out) with bias bq+bv; fp8 copy for Q'
                st[i] = chunk_tiles()
                q0 = i * QC
                for d in range(CCH):
                    qp = ps.tile([128, QC], F32, name="qp", tag="pR", bufs=2)
                    for cc in range(CCH):
                        nc.tensor.matmul(qp[:],
                                         lhsT=wq_b[:, cc, d * 128:(d + 1) * 128],
                                         rhs=x1t[:, cc, q0:q0 + QC],
                                         start=(cc == 0), stop=(cc == CCH - 1))
                    nc.vector.tensor_add(out=st[i]["qtf"][d][:], in0=qp[:],
                                         in1=bq_t[d][:].broadcast_to([128, QC]))
                    nc.scalar.activation(st[i]["qt8"][:, d, :], qp[:], AF.Identity,
                                         bias=bq_t[d][:])

            def emit_qprime(i):
                # Q'^T[c,q] = sum_d Wk[d,c] Q^T[d,q]  (fp8 DR over d-pairs)
                for cch in range(CCH):
                    pp = ps.tile([128, QC], F32, name="qpp", tag="pR", bufs=2)
                    for j2 in range(2):
                        nc.tensor.matmul(
                            pp[:],
                            lhsT=wkp[:, 2 * j2:2 * j2 + 2, cch * 128:(cch + 1) * 128],
                            rhs=st[i]["qt8"][:, 2 * j2:2 * j2 + 2, :],
                            start=(j2 == 0), stop=(j2 == 1), perf_mode=DR)
                    nc.scalar.activation(st[i]["qp8"][:, cch, :], pp[:], AF.Copy)

            def emit_s_kt(i, kt):
                # S^T[k-tile, q] = sum_c x2^T[c,k] Q'^T[c,q]; exp -> fp8
                sp = ps.tile([128, QC], F32, name="sps", tag="pA", bufs=2)
                for j2 in range(2):
                    nc.tensor.matmul(
                        sp[:],
                        lhsT=x2t[:, 2 * j2:2 * j2 + 2, kt * 128:(kt + 1) * 128],
                        rhs=st[i]["qp8"][:, 2 * j2:2 * j2 + 2, :],
                        start=(j2 == 0), stop=(j2 == 1), perf_mode=DR)
                nc.scalar.activation(st[i]["pt8"][kt // 2][:, kt % 2, :], sp[:],
                                     AF.Exp, scale=float(SCALE))

            def emit_r_j(i, j, rs):
                nc.tensor.matmul(rs[:], lhsT=s16[:], rhs=st[i]["pt8"][j][:],
                                 start=(j == 0), stop=(j == KT // 2 - 1),
                                 perf_mode=DR)

            def emit_ap_j(i, j, ap):
                # A'^T[c,q] += x2[k,c-block]^T-pairs . P^T[k,q]
                for cch in range(CCH):
                    nc.tensor.matmul(ap[cch // 2][:, cch % 2, :],
                                     lhsT=x2p[:, 2 * j:2 * j + 2,
                                              cch * 128:(cch + 1) * 128],
                                     rhs=st[i]["pt8"][j][:],
                                     start=(j == 0), stop=(j == KT // 2 - 1),
                                     perf_mode=DR)

            LAG = 3

            def emit_o_rb(i, rb):
                op = ps.tile([128, C], F32, name="ops", tag="pR", bufs=2)
                for d in range(CCH):
                    nc.tensor.matmul(
                        op[:],
                        lhsT=st[i]["u"][d][:, rb * 128:(rb + 1) * 128],
                        rhs=wo_t[d][:],
                        start=(d == 0), stop=(d == CCH - 1))
                ot = sb.tile([128, C], F32, name="ot", tag="ot", bufs=3)
                nc.vector.tensor_add(out=ot[:], in0=op[:], in1=bo_bc[:])
                nc.sync.dma_start(out=out_d[i * 4 + rb, :, :], in_=ot[:])

            def emit_chunk_core(i, o_prev=None):
                # S loop with rowsum/A' groups lagged in behind the exps;
                # the previous chunk's output projection fills the lag prefix
                rs = ps.tile([128, QC], F32, name="rs", tag="pR", bufs=2)
                ap = [ps.tile([128, 2, QC], F32, name="aps", tag="pB", bufs=2)
                      for _ in range(CCH)]
                for j in range(KT // 2):
                    emit_s_kt(i, 2 * j)
                    emit_s_kt(i, 2 * j + 1)
                    if o_prev is not None and 1 <= j <= 4:
                        emit_o_rb(o_prev, j - 1)
                    if j >= LAG:
                        emit_r_j(i, j - LAG, rs)
                        emit_ap_j(i, j - LAG, ap)
                for j in range(KT // 2 - LAG, KT // 2):
                    emit_r_j(i, j, rs)
                    emit_ap_j(i, j, ap)
                # scaled fp8 copies of A' (x0.25 keeps it inside fp8 range;
                # the scales in rs and a8 cancel through recip) — one wide
                # ACT op per 2-bank accumulator halves the cast overhead
                for h in range(2):
                    nc.scalar.activation(st[i]["a8"][:, 2 * h:2 * h + 2, :],
                                         ap[h][:], AF.Copy,
                                         scale=float(A_SCALE))
                nc.vector.reciprocal_approx_fast(out=st[i]["recip"][:], in_=rs[:])

            def emit_att_u(i):
                # att^T[d,q] = sum_c Wv^T[c,d] A'^T[c,q]; U = Q + att*recip
                for h in range(2):
                    pp = ps.tile([128, 2, QC], F32, name="attp", tag="pB",
                                 bufs=2)
                    for i2 in range(2):
                        d = 2 * h + i2
                        for j2 in range(2):
                            nc.tensor.matmul(
                                pp[:, i2, :],
                                lhsT=wv8[:, 2 * j2:2 * j2 + 2,
                                         d * 128:(d + 1) * 128],
                                rhs=st[i]["a8"][:, 2 * j2:2 * j2 + 2, :],
                                start=(j2 == 0), stop=(j2 == 1), perf_mode=DR)
                    for i2 in range(2):
                        d = 2 * h + i2
                        at = sb.tile([128, QC], F32, name="at", tag="at",
                                     bufs=2)
                        nc.vector.tensor_mul(out=at[:], in0=pp[:, i2, :],
                                             in1=st[i]["recip"][:])
                        nc.vector.tensor_add(out=st[i]["u"][d][:], in0=at[:],
                                             in1=st[i]["qtf"][d][:])

            # ---- schedule ----
            # PE warmup during the DMA lead-in (clock ramp + covers the
            # x1/wq transfer before Q0 can start)
            def emit_warm(n):
                for w in range(n):
                    wp = ps.tile([128, QC], F32, name="warmp", tag="pA", bufs=2)
                    nc.tensor.matmul(wp[:], lhsT=s16[:], rhs=warm[:],
                                     start=True, stop=True, perf_mode=DR)

            emit_warm(11)
            emit_q(0)
            emit_qprime(0)
            emit_warm(3)
            for i in range(NQC):
                emit_chunk_core(i, o_prev=(i - 1 if i > 0 else None))
                if i == 0:
                    for d in range(CCH):
                        nc.gpsimd.tensor_copy(out=wo_t[d][:],
                                              in_=wo_stage[:, d, :])
                if i + 1 < NQC:
                    emit_q(i + 1)
                emit_att_u(i)
                if i + 1 < NQC:
                    emit_qprime(i + 1)
            for rb in range(QC // 128):
                emit_o_rb(NQC - 1, rb)

    nc.compile()
    return nc


def get_built():
    global _BUILT
    if _BUILT is None:
        _BUILT = build()
    return _BUILT


def _pair_layout(a, dt):
    # [512 (contract), X] -> [128, 4, X]: [p, j, x] = a[j*128+p, x]
    return np.ascontiguousarray(
        a.reshape(CCH, 128, -1).transpose(1, 0, 2)).astype(dt)


def make_in_maps(x1, x2, Wq, bq, Wk, bk, Wv, bv, Wo, bo):
    bf = ml_dtypes.bfloat16
    f8 = ml_dtypes.float8_e4m3
    wq8 = _pair_layout(np.ascontiguousarray(Wq.T), bf)
    wkp = _pair_layout(np.ascontiguousarray(Wk), f8)
    wv8 = _pair_layout(np.ascontiguousarray(Wv.T), f8)
    wo8 = _pair_layout(np.ascontiguousarray(Wo.T), np.float32)
    # bv folds into the Q bias (rs*recip == 1); bk cancels in softmax
    bqv = (bq + bv).astype(np.float32)
    bq32 = np.ascontiguousarray(bqv.reshape(CCH, 128).T).astype(np.float32)
    bo32 = bo.astype(np.float32)
    x2t8 = [_pair_layout(np.ascontiguousarray(x2[b].T), f8) for b in range(B)]
    x2p8 = [np.ascontiguousarray(
        x2[b].reshape(KT, 128, C).transpose(1, 0, 2)).astype(f8)
        for b in range(B)]
    in_maps = []
    for cid in range(NCORES):
        b, h = cid // 2, cid % 2
        x1s = x1[b, h * QROWS:(h + 1) * QROWS, :]
        in_maps.append({
            "x1t": _pair_layout(np.ascontiguousarray(x1s.T), bf),
            "x2t": x2t8[b], "x2p": x2p8[b],
            "wq8": wq8, "wkp": wkp, "wv8": wv8, "wo8": wo8,
            "bq": bq32, "bo": bo32,
        })
    return in_maps


LAST_RESULT = None


def kernel(x1, x2, Wq, bq, Wk, bk, Wv, bv, Wo, bo):
    global LAST_RESULT
    nc = get_built()
    in_maps = make_in_maps(x1, x2, Wq, bq, Wk, bk, Wv, bv, Wo, bo)
    trace = bool(os.environ.get("KERNEL_TRACE"))
    res = run_bass_kernel_spmd(nc, in_maps, core_ids=list(range(NCORES)), trace=trace)
    LAST_RESULT = res
    out = np.empty((B, N1, C), dtype=np.float32)
    for cid in range(NCORES):
        b, h = cid // 2, cid % 2
        out[b, h * QROWS:(h + 1) * QROWS, :] = \
            res.results[cid]["out"].reshape(QROWS, C)
    return out
